# revision 2
# baseline (speedup 1.0000x reference)
"""Trainium2 Bass kernel v2 for nn_ChatbotModel.

Sharding: 8 cores = 4 batch-groups (16 batches each) x 2 vocab halves.
Every core runs the full encoder/decoder recurrence for its 16 batches
(bf16 matmuls, single-pass) and projects its vocab half of the logits.

Device-side tricks:
  - all matmuls bf16 (no fp32 LOW/HIGH double pass, FWL weight loads)
  - biases injected into PSUM via tiny rank-2/3 matmuls -> single wide ACT
  - sigmoid via 0.5+0.5*tanh(0.5x); GRU elementwise via scalar_tensor_tensor
    (4 DVE ops per cell)
  - W_attn folded into the next-step GRU weights (A1/A2/C1/C2) so the
    attention output `na` leaves the critical chain
  - attention scores via per-chunk [128,128]-stationary matmuls; softmax
    denominator via a batch-selector matmul; context via selector-scaled
    matmuls against the (constant) transposed memory
  - vocab projection packs 2x512-col chunks into 2 PSUM banks, one wide
    PSUM->SBUF masked copy, DMA out per 64-row group
"""

import os
import sys

for _p in ("/opt/trn_rl_repo", "/root/.axon_site/_ro/trn_rl_repo"):
    if os.path.isdir(_p) and _p not in sys.path:
        sys.path.insert(0, _p)

import numpy as np

import concourse.bacc as bacc
import concourse.mybir as mybir
import concourse.tile as tile
from concourse.bass_utils import run_bass_kernel_spmd

B, S, H, V, E = 64, 20, 128, 50257, 300
NCORES = 8
BG, VG = 4, 2                 # batch groups x vocab groups
BL = B // BG                  # 16 batches per core
SB = S * BL                   # 320 (s,b) pairs
NCH = (SB + 127) // 128       # 3 chunks; rows per chunk:
ROWS_C = [min(128, SB - 128 * c) for c in range(NCH)]   # [128,128,64]
VSH = (V + VG - 1) // VG      # 25129
VCP = ((VSH + 511) // 512) * 512   # 25600 padded
NPACK = VCP // 1024           # 25 packs of 1024 cols per row chunk
BIG = 100.0
NEG = -1e30

F32 = mybir.dt.float32
BF16 = mybir.dt.bfloat16
NPBF16 = mybir.dt.np(BF16)
AF = mybir.ActivationFunctionType
OP = mybir.AluOpType
AX = mybir.AxisListType

LAST_RESULT = None

def _build_nc():
    nc = bacc.Bacc("TRN2", target_bir_lowering=False, debug=False,
                   num_devices=NCORES)
    d = {}

    def din(name, shape, dt=BF16):
        d[name] = nc.dram_tensor(name, list(shape), dt, kind="ExternalInput").ap()

    # embeddings, pre-gathered/transposed to [E, (t,b)], bf16, 128-row chunks
    for nm in ("ex", "ey"):
        din(f"{nm}0", (128, SB)); din(f"{nm}1", (128, SB)); din(f"{nm}2", (44, SB))
    # x-part weight chunks (lhsT; contraction rows match emb chunks)
    for nm, g in (("e0gx", 256), ("e0cx", 128), ("d0gx", 256), ("d0cx", 128)):
        din(f"{nm}0", (128, g)); din(f"{nm}1", (128, g)); din(f"{nm}2", (44, g))
    # recurrent weights (bf16; wc* rh-halves pre-halved on host)
    din("wg0h", (128, 256)); din("wc0h", (128, 128))
    din("wg1t", (128, 256)); din("wg1b", (128, 256))
    din("wc1t", (128, 128)); din("wc1b", (128, 128))
    din("wgdh", (128, 256)); din("wcdh", (128, 128))
    din("a1", (128, 256)); din("a2", (128, 256))
    din("c1", (128, 128)); din("c2", (128, 128))
    din("wgd1t", (128, 256)); din("wgd1b", (128, 256))
    din("wcd1t", (128, 128)); din("wcd1b", (128, 128))
    # bias-injection tables
    din("bbe", (3, 128)); din("re", (3, S * 2 * BL))
    din("bbd", (2, 128)); din("rd", (2, 2 * BL))
    # biases as fp32 [128,1] (used as ACT bias in the x-part precompute)
    for nm in ("bg0r", "bg0u", "cb0", "bgyr", "bgyu", "cby", "cb1", "cbd1"):
        din(nm, (128, 1), F32)
    # attention
    din("wmem", (128, 128)); din("wq", (128, 128)); din("vcol", (128, 1))
    din("wat", (128, 128)); din("wab", (128, 128))
    # constants / masks
    din("i128", (128, 128)); din("onesr", (1, 128)); din("bigx", (1, SB))
    din("selbig", (128, 128)); din("selb", (128, BL)); din("mask2", (128, NCH))
    din("ym2", (128, NCH), F32)
    # vocab projection shard
    din("w16", (128, VCP))

    out = nc.dram_tensor("out", [SB, VCP], BF16, kind="ExternalOutput").ap()
    DBG = os.environ.get("KDEBUG", "0") == "1"
    dbg = {}
    if DBG:
        for nm, shp in (("dbg_memT2", [128, SB]), ("dbg_mem2", [128, NCH * 128]),
                        ("dbg_keysT2", [128, SB]), ("dbg_na", [128, SB]),
                        ("dbg_henc", [128, 2 * BL])):
            dbg[nm] = nc.dram_tensor(nm, shp, BF16, kind="ExternalOutput").ap()

    from contextlib import ExitStack
    with tile.TileContext(nc) as tc, ExitStack() as ctx:
        const = ctx.enter_context(tc.tile_pool(name="const", bufs=1))
        big = ctx.enter_context(tc.tile_pool(name="big", bufs=1))
        work = ctx.enter_context(tc.tile_pool(name="work", bufs=3))
        st = ctx.enter_context(tc.tile_pool(name="st", bufs=4))
        outp = ctx.enter_context(tc.tile_pool(name="outp", bufs=4))
        ps = ctx.enter_context(tc.tile_pool(name="ps", bufs=4, space="PSUM"))
        pj = ctx.enter_context(tc.tile_pool(name="pj", bufs=2, space="PSUM"))

        cst = {}

        def load(name, pool=const):
            ap = d[name]
            t = pool.tile(list(ap.shape), ap.dtype, tag=name, name=name)
            nc.sync.dma_start(t[:], ap[:])
            cst[name] = t
            return t

        # encoder-critical first, w16 last (not needed until decode t>=4)
        for nm in ("i128", "onesr", "bigx",
                   "e0gx0", "e0gx1", "e0gx2", "e0cx0", "e0cx1", "e0cx2",
                   "ex0", "ex1", "ex2",
                   "bg0r", "bg0u", "cb0",
                   "wg0h", "wc0h", "wg1t", "wg1b", "wc1t", "wc1b",
                   "bbe", "re", "cb1",
                   "d0gx0", "d0gx1", "d0gx2", "d0cx0", "d0cx1", "d0cx2",
                   "ey0", "ey1", "ey2", "bgyr", "bgyu", "cby",
                   "wmem", "wq", "vcol", "wat", "wab",
                   "wgdh", "wcdh", "a1", "a2", "c1", "c2",
                   "wgd1t", "wgd1b", "wcd1t", "wcd1b", "bbd", "rd", "cbd1",
                   "selbig", "selb", "mask2", "ym2",
                   "w16"):
            load(nm, big if nm == "w16" else const)

        i128 = cst["i128"]
        MM = nc.tensor.matmul

        # ---------------- x-part precompute ----------------
        # xg*ru: [128, S*32] bf16, per-t slice = [r(16) | u(16)], biases baked
        # (u-half of the encoder additionally gets +BIG at invalid steps).
        xg0ru = big.tile([128, S * 2 * BL], BF16, tag="xg0ru")
        yg0ru = big.tile([128, S * 2 * BL], BF16, tag="yg0ru")
        xc0 = big.tile([128, SB], BF16, tag="xc0")
        yc0 = big.tile([128, SB], BF16, tag="yc0")

        def xpart_g(dstv, wpre, srcs, br, bu, add_big):
            for half, bias in ((0, br), (1, bu)):
                p = ps.tile([128, SB], F32, tag="ps")
                g0 = half * 128
                with_big = add_big and half == 1
                for k in range(3):
                    MM(p[:], cst[f"{wpre}{k}"][:, g0:g0 + 128], cst[srcs[k]][:],
                       start=(k == 0), stop=(k == 2 and not with_big),
                       skip_group_check=(k > 0))
                if with_big:
                    MM(p[:], cst["onesr"][:], cst["bigx"][:],
                       start=False, stop=True, skip_group_check=True)
                dstap = dstv[:].rearrange("p (s w) -> p s w", w=2 * BL)[
                    :, :, half * BL:(half + 1) * BL]
                nc.scalar.activation(
                    dstap, p[:].rearrange("p (s b) -> p s b", b=BL),
                    AF.Identity, bias=cst[bias][:])

        def xpart_c(dstv, wpre, srcs, cb):
            p = ps.tile([128, SB], F32, tag="ps")
            for k in range(3):
                MM(p[:], cst[f"{wpre}{k}"][:], cst[srcs[k]][:],
                   start=(k == 0), stop=(k == 2), skip_group_check=(k > 0))
            nc.scalar.activation(dstv[:], p[:], AF.Identity, bias=cst[cb][:])

        exs = ("ex0", "ex1", "ex2"); eys = ("ey0", "ey1", "ey2")
        xpart_g(xg0ru, "e0gx", exs, "bg0r", "bg0u", True)
        xpart_c(xc0, "e0cx", exs, "cb0")
        xpart_g(yg0ru, "d0gx", eys, "bgyr", "bgyu", False)
        xpart_c(yc0, "d0cx", eys, "cby")

        # ---------------- GRU cell (bf16) ----------------
        stt = nc.vector.scalar_tensor_tensor

        def cell_tail(tag, G, h_in, c_terms, cb, h_out_ap):
            """From completed G-psum: tg=tanh(0.5 G); rh=(tg_r+1)h;
            C = c_terms + wc_rh@rh; c=tanh(C+cb); h'=c+0.5(tg_u+1)(h-c)."""
            tg = work.tile([128, 2 * BL], BF16, tag="tg" + tag)
            nc.scalar.activation(tg[:], G[:], AF.Tanh, scale=0.5)
            rh = work.tile([128, BL], BF16, tag="rh" + tag)
            stt(rh[:], tg[:, 0:BL], 1.0, h_in, OP.add, OP.mult)
            C = ps.tile([128, BL], F32, tag="ps")
            for i, (l, r_) in enumerate(c_terms):
                MM(C[:], l, r_, start=(i == 0), stop=False,
                   skip_group_check=(i > 0))
            MM(C[:], c_terms_rh[tag][:], rh[:],
               start=False, stop=True, skip_group_check=True)
            c_sb = work.tile([128, BL], BF16, tag="c" + tag)
            if cb is None:
                nc.scalar.activation(c_sb[:], C[:], AF.Tanh)
            else:
                nc.scalar.activation(c_sb[:], C[:], AF.Tanh, bias=cb[:])
            dd = work.tile([128, BL], BF16, tag="d" + tag)
            nc.vector.tensor_sub(dd[:], h_in, c_sb[:])
            ee = work.tile([128, BL], BF16, tag="e" + tag)
            stt(ee[:], tg[:, BL:2 * BL], 1.0, dd[:], OP.add, OP.mult)
            stt(h_out_ap, ee[:], 0.5, c_sb[:], OP.mult, OP.add)

        c_terms_rh = {}   # tag -> wc weight tile for the rh term

        # ---------------- encoder ----------------
        h0 = st.tile([128, BL], BF16, tag="h0")
        nc.vector.memset(h0[:], 0.0)
        hm1 = st.tile([128, BL], BF16, tag="hm1")
        nc.vector.memset(hm1[:], 0.0)
        memT2 = big.tile([128, SB], BF16, tag="memT2")     # h1 states, [h,(t,b)]
        mem2 = big.tile([128, NCH * 128], BF16, tag="mem2")  # [(s8,b), h] chunks

        c_terms_rh["e0"] = cst["wc0h"]
        c_terms_rh["e1"] = cst["wc1b"]

        def enc_l0(t):
            nonlocal h0
            G = ps.tile([128, 2 * BL], F32, tag="ps")
            sl = slice(t * 2 * BL, (t + 1) * 2 * BL)
            MM(G[:], i128[:], xg0ru[:, sl], start=True, stop=False)
            MM(G[:, 0:BL], cst["wg0h"][:, 0:128], h0[:],
               start=False, stop=False, skip_group_check=True)
            MM(G[:, BL:2 * BL], cst["wg0h"][:, 128:256], h0[:],
               start=False, stop=True, skip_group_check=True)
            h_new = st.tile([128, BL], BF16, tag="h0")
            csl = slice(t * BL, (t + 1) * BL)
            cell_tail("e0", G, h0[:], [(i128[:], xc0[:, csl])], None, h_new[:])
            h0 = h_new

        def enc_l1(j):
            # h1(j) written into memT2[:, j*BL:(j+1)*BL]
            h1p = hm1[:] if j == 0 else memT2[:, (j - 1) * BL:j * BL]
            G = ps.tile([128, 2 * BL], F32, tag="ps")
            MM(G[:], cst["bbe"][:], cst["re"][:, j * 2 * BL:(j + 1) * 2 * BL],
               start=True, stop=False)
            MM(G[:, 0:BL], cst["wg1b"][:, 0:128], h1p,
               start=False, stop=False, skip_group_check=True)
            MM(G[:, BL:2 * BL], cst["wg1b"][:, 128:256], h1p,
               start=False, stop=False, skip_group_check=True)
            MM(G[:, 0:BL], cst["wg1t"][:, 0:128], h0e[j][:],
               start=False, stop=False, skip_group_check=True)
            MM(G[:, BL:2 * BL], cst["wg1t"][:, 128:256], h0e[j][:],
               start=False, stop=True, skip_group_check=True)
            cell_tail("e1", G, h1p, [(cst["wc1t"][:], h0e[j][:])], cst["cb1"],
                      memT2[:, j * BL:(j + 1) * BL])

        h0e = {}
        for t in range(S):
            if t > 0:
                enc_l1(t - 1)
                h0e.pop(t - 1, None)
            enc_l0(t)
            h0e[t] = h0
            if t >= 1 and (t - 1) % 8 == 7:
                cc = (t - 1) // 8
                trp = ps.tile([128, 128], BF16, tag="ps")
                nc.tensor.transpose(trp[:], memT2[:, cc * 128:(cc + 1) * 128],
                                    i128[:])
                nc.scalar.copy(mem2[:, cc * 128:(cc + 1) * 128], trp[:])
        enc_l1(S - 1)
        # last partial chunk: [128, 64] -> [64, 128]
        trp = ps.tile([128, 128], BF16, tag="ps")
        nc.tensor.transpose(trp[0:64, :], memT2[:, 256:SB], i128[:])
        nc.scalar.copy(mem2[0:64, 2 * 128:3 * 128], trp[0:64, :])

        h1f = memT2[:, (S - 1) * BL:S * BL]

        if DBG:
            nc.sync.dma_start(dbg["dbg_memT2"][:], memT2[:])
            nc.sync.dma_start(dbg["dbg_mem2"][:], mem2[:])
            henc = work.tile([128, 2 * BL], BF16, tag="henc")
            nc.vector.tensor_copy(henc[:, 0:BL], h0[:])
            nc.vector.tensor_copy(henc[:, BL:2 * BL], h1f)
            nc.sync.dma_start(dbg["dbg_henc"][:], henc[:])

        # keysT2 = W_mem.T @ memT2
        kp = ps.tile([128, SB], F32, tag="ps")
        MM(kp[:], cst["wmem"][:], memT2[:], start=True, stop=True)
        keysT2 = big.tile([128, SB], BF16, tag="keysT2")
        nc.scalar.copy(keysT2[:], kp[:])
        if DBG:
            nc.sync.dma_start(dbg["dbg_keysT2"][:], keysT2[:])

        # ---------------- decoder ----------------
        naT16 = big.tile([128, SB], BF16, tag="naT16")
        c_terms_rh["d0"] = cst["wcdh"]
        c_terms_rh["d1"] = cst["wcd1b"]

        h1 = None      # h1(t-1) AP; None -> h1f
        ctx16 = None   # ctx(t-1) tile

        def open_G0(t, h0p, h1p):
            G = ps.tile([128, 2 * BL], F32, tag="ps")
            sl = slice(t * 2 * BL, (t + 1) * 2 * BL)
            MM(G[:], i128[:], yg0ru[:, sl], start=True, stop=False)
            MM(G[:, 0:BL], cst["wgdh"][:, 0:128], h0p,
               start=False, stop=False, skip_group_check=True)
            MM(G[:, BL:2 * BL], cst["wgdh"][:, 128:256], h0p,
               start=False, stop=(t == 0), skip_group_check=True)
            if t > 0:
                MM(G[:, 0:BL], cst["a1"][:, 0:128], h1p,
                   start=False, stop=False, skip_group_check=True)
                MM(G[:, BL:2 * BL], cst["a1"][:, 128:256], h1p,
                   start=False, stop=False, skip_group_check=True)
            return G

        def open_G1(h1p):
            G = ps.tile([128, 2 * BL], F32, tag="ps")
            MM(G[:], cst["bbd"][:], cst["rd"][:], start=True, stop=False)
            MM(G[:, 0:BL], cst["wgd1b"][:, 0:128], h1p,
               start=False, stop=False, skip_group_check=True)
            MM(G[:, BL:2 * BL], cst["wgd1b"][:, 128:256], h1p,
               start=False, stop=False, skip_group_check=True)
            return G

        # projection pack queue: (rowchunk, packidx); chunk rc covers rows
        # rc*128 .. rc*128+ROWS_C[rc] (t = 8rc .. 8rc+7)
        pending = []

        def emit_pack(rc, p):
            rows = ROWS_C[rc]
            na_sl = naT16[:, rc * 128:rc * 128 + rows]
            c0 = p * 1024
            pp = pj.tile([128, 1024], F32, tag="pp")
            w = cst["w16"]
            MM(pp[0:rows, 0:512], na_sl, w[:, c0:c0 + 512],
               start=True, stop=False)
            MM(pp[0:rows, 512:1024], na_sl, w[:, c0 + 512:c0 + 1024],
               start=True, stop=True, skip_group_check=True)
            ot = outp.tile([128, 1024], BF16, tag="ot")
            ymc = cst["ym2"][:, rc:rc + 1]
            if p % 2 == 0:
                nc.scalar.mul(ot[0:rows, :], pp[0:rows, :], ymc[0:rows, :])
            else:
                nc.vector.tensor_scalar_mul(ot[0:rows, :], pp[0:rows, :],
                                            ymc[0:rows, :])
            rsl = slice(rc * 128, rc * 128 + rows)
            nc.sync.dma_start(out[rsl, c0:c0 + 1024], ot[0:rows, :])

        G0 = open_G0(0, h0[:], None)
        G1 = open_G1(h1f)

        for t in range(S):
            h1p = h1f if t == 0 else h1[:]
            # ---- close & run L0(t) ----
            if t > 0:
                MM(G0[:, 0:BL], cst["a2"][:, 0:128], ctx16[:],
                   start=False, stop=False, skip_group_check=True)
                MM(G0[:, BL:2 * BL], cst["a2"][:, 128:256], ctx16[:],
                   start=False, stop=True, skip_group_check=True)
            csl = slice(t * BL, (t + 1) * BL)
            c_terms = [(i128[:], yc0[:, csl])]
            if t > 0:
                c_terms.append((cst["c1"][:], h1p))
                c_terms.append((cst["c2"][:], ctx16[:]))
            h0n = st.tile([128, BL], BF16, tag="h0")
            cell_tail("d0", G0, h0[:], c_terms, None, h0n[:])
            h0 = h0n
            # ---- close & run L1(t) ----
            MM(G1[:, 0:BL], cst["wgd1t"][:, 0:128], h0[:],
               start=False, stop=False, skip_group_check=True)
            MM(G1[:, BL:2 * BL], cst["wgd1t"][:, 128:256], h0[:],
               start=False, stop=True, skip_group_check=True)
            h1n = st.tile([128, BL], BF16, tag="h1")
            cell_tail("d1", G1, h1p, [(cst["wcd1t"][:], h0[:])], cst["cbd1"],
                      h1n[:])
            h1 = h1n
            # ---- open next step's G groups (fills PE during attention) ----
            if t + 1 < S:
                G0 = open_G0(t + 1, h0[:], h1[:])
                G1 = open_G1(h1[:])
            # ---- attention ----
            qP = ps.tile([128, BL], F32, tag="ps")
            MM(qP[:], cst["wq"][:], h1[:], start=True, stop=True)
            pre = work.tile([128, SB], BF16, tag="pre")
            nc.vector.tensor_add(
                pre[:].rearrange("p (s b) -> p s b", b=BL),
                keysT2[:].rearrange("p (s b) -> p s b", b=BL),
                qP[:, None, :].broadcast_to((128, S, BL)))
            th = work.tile([128, SB], BF16, tag="th")
            nc.scalar.activation(th[:], pre[:], AF.Tanh)
            sc = ps.tile([128, NCH], F32, tag="ps")
            for c in range(NCH):
                rc = ROWS_C[c]
                MM(sc[0:rc, c:c + 1], th[:, c * 128:c * 128 + rc],
                   cst["vcol"][:], start=(c == 0), stop=False,
                   skip_group_check=(c > 0))
            MM(sc[:], i128[:], cst["mask2"][:], start=False, stop=True,
               skip_group_check=True)
            exp2 = work.tile([128, NCH], BF16, tag="exp2")
            nc.scalar.activation(exp2[:], sc[:], AF.Exp)
            denP = ps.tile([128, NCH], F32, tag="ps")
            MM(denP[:], cst["selbig"][:], exp2[:], start=True, stop=True)
            den = work.tile([128, 1], F32, tag="den")
            nc.vector.reduce_sum(den[:], denP[:], axis=AX.X)
            rec = work.tile([128, 1], F32, tag="rec")
            nc.vector.reciprocal(rec[:], den[:])
            wts = work.tile([128, NCH], F32, tag="wts")
            nc.vector.tensor_scalar_mul(wts[:], exp2[:], rec[:])
            selw = work.tile([128, NCH * BL], BF16, tag="selw")
            for c in range(NCH):
                rc = ROWS_C[c]
                nc.vector.tensor_scalar_mul(
                    selw[0:rc, c * BL:(c + 1) * BL], cst["selb"][0:rc, :],
                    wts[0:rc, c:c + 1])
            ctxP = ps.tile([128, BL], F32, tag="ps")
            for c in range(NCH):
                rc = ROWS_C[c]
                MM(ctxP[:], mem2[0:rc, c * 128:(c + 1) * 128],
                   selw[0:rc, c * BL:(c + 1) * BL],
                   start=(c == 0), stop=(c == NCH - 1),
                   skip_group_check=(c > 0))
            ctxn = st.tile([128, BL], BF16, tag="ctx")
            nc.scalar.copy(ctxn[:], ctxP[:])
            ctx16 = ctxn
            # ---- na (off-chain; feeds projection only) ----
            naP = ps.tile([128, BL], F32, tag="ps")
            MM(naP[:], cst["wat"][:], h1[:], start=True, stop=False)
            MM(naP[:], cst["wab"][:], ctx16[:], start=False, stop=True,
               skip_group_check=True)
            nc.vector.tensor_copy(naT16[:, t * BL:(t + 1) * BL], naP[:])
            # ---- projection packs ----
            if t == 8 or t == 16:
                pending.extend((t // 8 - 1, p) for p in range(NPACK))
            npop = min(len(pending), 4 if t < 16 else 7)
            for _ in range(npop):
                emit_pack(*pending.pop(0))

        if DBG:
            nc.sync.dma_start(dbg["dbg_na"][:], naT16[:])

        # tail: last row chunk + any leftovers
        pending.extend((NCH - 1, p) for p in range(NPACK))
        while pending:
            emit_pack(*pending.pop(0))

    nc.compile()
    return nc


_NC_CACHE = None


def _get_nc():
    global _NC_CACHE
    if _NC_CACHE is None:
        _NC_CACHE = _build_nc()
    return _NC_CACHE


def _host_prep(inp):
    f32 = np.float32
    x = np.asarray(inp["x"]); y = np.asarray(inp["y"])
    xl = np.asarray(inp["x_length"]); yl = np.asarray(inp["y_length"])
    emb = np.asarray(inp["embedding"], f32)
    g = lambda k: np.asarray(inp[k], f32)
    b16 = lambda a: np.ascontiguousarray(a).astype(NPBF16)

    m = {}   # shared (core-independent) tensors
    e0_gk, e0_ck = g("e0_gk"), g("e0_ck")
    d0_gk, d0_ck = g("d0_gk"), g("d0_ck")
    for nm, w in (("e0gx", e0_gk), ("e0cx", e0_ck),
                  ("d0gx", d0_gk), ("d0cx", d0_ck)):
        m[f"{nm}0"], m[f"{nm}1"], m[f"{nm}2"] = \
            b16(w[0:128]), b16(w[128:256]), b16(w[256:300])
    m["wg0h"] = b16(e0_gk[300:428])
    m["wc0h"] = b16(0.5 * e0_ck[300:428])
    e1_gk, e1_ck = g("e1_gk"), g("e1_ck")
    m["wg1t"], m["wg1b"] = b16(e1_gk[0:128]), b16(e1_gk[128:256])
    m["wc1t"], m["wc1b"] = b16(e1_ck[0:128]), b16(0.5 * e1_ck[128:256])
    m["wgdh"] = b16(d0_gk[428:556])
    m["wcdh"] = b16(0.5 * d0_ck[428:556])
    W_attn = g("W_attn")
    wgdna, wcdna = d0_gk[300:428], d0_ck[300:428]
    m["a1"] = b16(W_attn[0:128] @ wgdna)
    m["a2"] = b16(W_attn[128:256] @ wgdna)
    m["c1"] = b16(W_attn[0:128] @ wcdna)
    m["c2"] = b16(W_attn[128:256] @ wcdna)
    d1_gk, d1_ck = g("d1_gk"), g("d1_ck")
    m["wgd1t"], m["wgd1b"] = b16(d1_gk[0:128]), b16(d1_gk[128:256])
    m["wcd1t"], m["wcd1b"] = b16(d1_ck[0:128]), b16(0.5 * d1_ck[128:256])

    col = lambda v: np.ascontiguousarray(v.reshape(128, 1)).astype(f32)
    e0_gb, e1_gb = g("e0_gb"), g("e1_gb")
    d0_gb, d1_gb = g("d0_gb"), g("d1_gb")
    m["bg0r"], m["bg0u"], m["cb0"] = col(e0_gb[0:128]), col(e0_gb[128:256]), col(g("e0_cb"))
    m["bgyr"], m["bgyu"], m["cby"] = col(d0_gb[0:128]), col(d0_gb[128:256]), col(g("d0_cb"))
    m["cb1"], m["cbd1"] = col(g("e1_cb")), col(g("d1_cb"))

    bbe = np.zeros((3, 128), f32)
    bbe[0] = e1_gb[0:128]; bbe[1] = e1_gb[128:256]; bbe[2] = 1.0
    m["bbe"] = b16(bbe)
    bbd = np.zeros((2, 128), f32)
    bbd[0] = d1_gb[0:128]; bbd[1] = d1_gb[128:256]
    m["bbd"] = b16(bbd)
    rd = np.zeros((2, 2 * BL), f32)
    rd[0, 0:BL] = 1.0; rd[1, BL:2 * BL] = 1.0
    m["rd"] = b16(rd)

    m["wmem"], m["wq"] = b16(g("W_mem")), b16(g("W_q"))
    m["vcol"] = b16(g("v_att").reshape(128, 1))
    m["wat"], m["wab"] = b16(W_attn[0:128]), b16(W_attn[128:256])
    m["i128"] = b16(np.eye(128, dtype=f32))
    m["onesr"] = b16(np.ones((1, 128), f32))
    p = np.arange(128)
    m["selbig"] = b16((p[:, None] % BL == p[None, :] % BL).astype(f32))
    m["selb"] = b16((p[:, None] % BL == np.arange(BL)[None, :]).astype(f32))

    W_proj = g("W_proj")

    in_maps = []
    for k in range(NCORES):
        gi, vi = k // VG, k % VG
        bsl = slice(gi * BL, (gi + 1) * BL)
        x_g, y_g = x[bsl], y[bsl]
        xl_g, yl_g = xl[bsl], yl[bsl]
        mk = dict(m)
        ex = emb[x_g]   # [BL, S, E]
        ey = emb[y_g]
        exT = np.ascontiguousarray(ex.transpose(2, 1, 0).reshape(E, SB))
        eyT = np.ascontiguousarray(ey.transpose(2, 1, 0).reshape(E, SB))
        mk["ex0"], mk["ex1"], mk["ex2"] = b16(exT[0:128]), b16(exT[128:256]), b16(exT[256:300])
        mk["ey0"], mk["ey1"], mk["ey2"] = b16(eyT[0:128]), b16(eyT[128:256]), b16(eyT[256:300])
        x_valid = (np.arange(S)[None, :] < xl_g[:, None])   # [BL, S]
        y_valid = (np.arange(S)[None, :] < yl_g[:, None])
        # bigx: [1, SB], col t*BL+b -> BIG at invalid steps
        mk["bigx"] = b16((BIG * (~x_valid).T.astype(f32)).reshape(1, SB))
        # re: [3, S*2*BL]: r-sel, u-sel, BIG*invalid on u cols
        re = np.zeros((3, S * 2 * BL), f32)
        for t in range(S):
            re[0, t * 2 * BL:t * 2 * BL + BL] = 1.0
            re[1, t * 2 * BL + BL:(t + 1) * 2 * BL] = 1.0
            re[2, t * 2 * BL + BL:(t + 1) * 2 * BL] = BIG * (~x_valid[:, t]).astype(f32)
        mk["re"] = b16(re)
        # mask2 [128, NCH]: chunk c, partition p -> (s=8c+p//BL, b=p%BL)
        mask2 = np.full((128, NCH), NEG, f32)
        for c in range(NCH):
            for pp_ in range(ROWS_C[c]):
                s_, b_ = 8 * c + pp_ // BL, pp_ % BL
                if s_ < S and x_valid[b_, s_]:
                    mask2[pp_, c] = 0.0
        mk["mask2"] = b16(mask2)
        # ym2 [128, NCH]: row chunk rc, partition p -> row rc*128+p
        ym2 = np.zeros((128, NCH), f32)
        for rc in range(NCH):
            for pp_ in range(ROWS_C[rc]):
                r_ = rc * 128 + pp_
                t_, b_ = r_ // BL, r_ % BL
                ym2[pp_, rc] = 1.0 if y_valid[b_, t_] else 0.0
        mk["ym2"] = ym2.astype(f32)
        # vocab shard
        wk = np.zeros((128, VCP), f32)
        ncols = min(VSH, V - vi * VSH)
        wk[:, :ncols] = W_proj[:, vi * VSH:vi * VSH + ncols]
        mk["w16"] = b16(wk)
        in_maps.append(mk)
    return in_maps


def kernel(**inputs):
    global LAST_RESULT
    nc = _get_nc()
    in_maps = _host_prep(inputs)
    res = run_bass_kernel_spmd(nc, in_maps, list(range(NCORES)))
    LAST_RESULT = res
    yl = np.asarray(inputs["y_length"])
    b_proj = np.asarray(inputs["b_proj"], np.float32)
    full = np.zeros((B, S, V), np.float32)
    for k in range(NCORES):
        gi, vi = k // VG, k % VG
        sh = np.asarray(res.results[k]["out"], dtype=NPBF16).astype(np.float32)
        ncols = min(VSH, V - vi * VSH)
        # row r = t*BL + b
        sh3 = sh[:, :ncols].reshape(S, BL, ncols).transpose(1, 0, 2)
        full[gi * BL:(gi + 1) * BL, :, vi * VSH:vi * VSH + ncols] = sh3
    full += b_proj[None, None, :]
    y_valid = (np.arange(S)[None, :] < yl[:, None])
    full[~y_valid] = 0.0
    return full


if __name__ == "__main__":
    _get_nc()
    print("built ok")
